# revision 19
# baseline (speedup 1.0000x reference)
"""CompGCN layer on 8 Trainium2 NeuronCores.

Reference computation:
    hn  = h * norm
    msg = (hn[src] - r[rel]) @ W_msg
    agg = segment_sum(msg, dst, N) * norm
    out = relu(hn @ W + agg + b)

Algebraic rewrite (matmul distributes over segment_sum):
    seg  = segment_sum(h[src]*norm[src], dst) - C @ r
    out  = relu(diag(norm) @ (h @ W + seg @ W_msg) + b)
where C[n,k] = #edges(dst=n, rel=k) is an integer histogram of the edge
structure (host-side index preprocessing, like the edge sort itself).
This turns the E x D x D per-edge matmul into an N x D x D one and the
scatter into one-hot matmuls accumulated in PSUM.

Sharding: edges are partitioned by 128-node destination windows; core i owns
49 consecutive windows and produces those output rows (no collectives).
h is replicated (as a bf16 table) so each core can gather arbitrary rows.

Device pipeline per 128-edge tile (edges pre-grouped by dst window on host):
    X  = dma_gather(hb, src)             # [128e, 128f] bf16 rows from HBM
    S  = (iota==dstl) * nsrc             # ONE fused DVE op -> scaled one-hot
    psum_w[f, p] += X.T @ S              # accumulates seg^T directly
Per-window epilogue (transpose-free: cmat/hwin shipped pre-transposed):
    psum_w  += (-r).T @ C^T chunks       # relation correction
    segT     = bf16(psum_w)              # one ACT copy
    op       = segT.T@Wm + hwinT.T@Wo + invnorm (x) b   # three matmuls
    out_w    = relu(norm * op)           # ACT with per-partition scale
(relu(norm*M + b) == relu(norm*(M + b/norm)) since norm > 0; the bias
enters as a rank-1 invnorm (x) b matmul, numerically safe as the 1/norm
rounding cancels when rescaled by norm.)
"""

import math
import numpy as np

from concourse import bass, bacc, mybir
from concourse import tile
from concourse.bass_utils import run_bass_kernel_spmd

FP32 = mybir.dt.float32
BF16 = mybir.dt.bfloat16
I16 = mybir.dt.int16

BF16_NP = np.dtype(mybir.dt.np(BF16))
FP8 = mybir.dt.float8e4
FP8_NP = np.dtype(mybir.dt.np(FP8))

P = 128          # partitions / window size / feature dim
N_CORES = 8


# ---------------------------------------------------------------------------
# Host-side preprocessing: index/layout work (sort, pad, wrap, integer
# histograms, dtype/layout prep of inputs). Per-edge scalar metadata
# (norm[src]) is gathered host-side; all math happens on device.
# ---------------------------------------------------------------------------

def _wrap16(idx_flat):
    """dma_gather index layout: i -> [partition i%16, col i//16], replicated
    to 128 partitions (8 Q7 cores each read one 16-row stripe)."""
    n = idx_flat.shape[0]
    assert n % 16 == 0
    w = idx_flat.reshape(n // 16, 16).T          # [16, n/16]
    return np.tile(w, (8, 1)).astype(np.int16)   # [128, n/16]


def _prep(h, r, norm, src, dst, rel, W_msg, W, b,
          n_cores=N_CORES, lo_split=32768, group_w=6, gchunk=8):
    N, D = h.shape
    R = r.shape[0]
    assert D == P
    RC = math.ceil(R / P)
    assert RC == 2

    NP_ = ((N + P - 1) // P) * P                 # padded node count
    n_win = NP_ // P                             # total windows
    wpc = (n_win + n_cores - 1) // n_cores       # windows per core (uniform)

    norm1 = np.asarray(norm).reshape(-1).astype(np.float32)
    src = np.asarray(src).astype(np.int64)
    dst = np.asarray(dst).astype(np.int64)
    rel = np.asarray(rel).astype(np.int64)

    win = dst // P                               # global window of each edge
    core = np.minimum(win // wpc, n_cores - 1)
    is_lo = src < lo_split

    # per-core per-window edge counts -> shared tile counts (max over cores)
    lo_cnt = np.zeros((n_cores, wpc), np.int64)
    hi_cnt = np.zeros((n_cores, wpc), np.int64)
    for c in range(n_cores):
        m = core == c
        wl = win[m] - c * wpc
        l = is_lo[m]
        np.add.at(lo_cnt[c], wl[l], 1)
        np.add.at(hi_cnt[c], wl[~l], 1)

    lo_tiles = np.maximum(1, np.ceil(lo_cnt.max(0) / P).astype(np.int64))
    hi_tiles = np.maximum(1, np.ceil(hi_cnt.max(0) / P).astype(np.int64))

    groups = [list(range(g, min(g + group_w, wpc)))
              for g in range(0, wpc, group_w)]

    # tile order (same for every core): per group, lo tiles then hi tiles.
    # gather chunks of <= gchunk tiles; a chunk never crosses the lo/hi
    # boundary (different tables).
    tile_order = []          # list of (window, is_lo)
    group_meta = []          # per group: (t0, ntiles, [(rel_t0, n, is_lo)])
    t = 0
    for ws in groups:
        t0 = t
        chunks = []
        for lo_flag, tcounts in ((True, lo_tiles), (False, hi_tiles)):
            seg0 = t
            for w in ws:
                for _ in range(int(tcounts[w])):
                    tile_order.append((w, lo_flag))
                    t += 1
            n_seg = t - seg0
            c = 0
            while c < n_seg:
                ce = min(c + gchunk, n_seg)
                chunks.append((seg0 - t0 + c, ce - c, lo_flag))
                c = ce
        group_meta.append((t0, t - t0, chunks))
    T = t                                       # total tiles per core
    gm = max(n for (_, n, _) in group_meta)     # max tiles per group

    struct = dict(N=N, NP=NP_, D=D, R=R, n_win=n_win, wpc=wpc,
                  lo_split=lo_split, groups=groups,
                  lo_tiles=[int(x) for x in lo_tiles],
                  hi_tiles=[int(x) for x in hi_tiles],
                  tile_order=tile_order, group_meta=group_meta,
                  T=T, gm=gm, gchunk=gchunk)

    h_np = np.asarray(h, np.float32)
    hb = np.zeros((NP_, D), BF16_NP)
    hb[:N] = h_np.astype(BF16_NP)                # bf16 gather table

    tile_ids = {}
    for ti, (w, lo) in enumerate(tile_order):
        tile_ids.setdefault((w, lo), []).append(ti)

    in_maps = []
    for c in range(n_cores):
        m = np.nonzero(core == c)[0]
        wl = win[m] - c * wpc
        slots_idx = np.zeros((T, P), np.int32)       # gather row index
        slots_dstl = np.full((T, P), P, np.float32)  # 128 sentinel -> S col off
        slots_nsrc = np.zeros((T, P), np.float32)

        fill = dict.fromkeys(tile_ids, 0)
        e_lo = is_lo[m]
        e_src = src[m]
        e_dstl = (dst[m] % P).astype(np.float32)
        e_nsrc = norm1[src[m]]
        for j in range(m.shape[0]):
            k = (int(wl[j]), bool(e_lo[j]))
            f = fill[k]
            ti = tile_ids[k][f // P]
            pos = f % P
            fill[k] = f + 1
            s = int(e_src[j])
            slots_idx[ti, pos] = s if e_lo[j] else s - lo_split
            slots_dstl[ti, pos] = e_dstl[j]
            slots_nsrc[ti, pos] = e_nsrc[j]

        idx_cols = []
        for (t0, ntt, chunks) in group_meta:
            for (rel_t0, n, _) in chunks:
                a = t0 + rel_t0
                idx_cols.append(_wrap16(slots_idx[a:a + n].reshape(-1)))
        idxw = np.concatenate(idx_cols, axis=1)      # [128, 8T]

        # integer (dst, rel) histogram for this core's windows, bf16-exact,
        # shipped pre-transposed: cmat_tk[k, w*128 + p] = C[w*128 + p, k']
        base = c * wpc * P
        cmat = np.zeros(wpc * P * R, np.int64)
        np.add.at(cmat, (dst[m] - base) * R + rel[m], 1)
        assert cmat.max() <= 16, "C counts exceed fp8-exact range"
        cmat = cmat.reshape(wpc * P, R)              # [wpc*128, R]
        cm_t0 = np.ascontiguousarray(cmat[:, :P].T.astype(FP8_NP))
        cm_t1 = np.ascontiguousarray(cmat[:, P:R].T.astype(FP8_NP))

        # own node rows shipped transposed: hwin_t[f, w*128+p] = h[base+...]
        own_n = min(max(N - base, 0), wpc * P)
        hwin = np.zeros((wpc * P, D), np.float32)
        nv = np.ones(wpc * P, np.float32)            # pad norm = 1 (finite inv)
        if own_n > 0:
            hwin[:own_n] = h_np[base:base + own_n]
            nv[:own_n] = norm1[base:base + own_n]
        hwin_t = np.ascontiguousarray(hwin.T.astype(FP8_NP))   # [128, wpc*128]
        nwin = np.ascontiguousarray(nv.reshape(wpc, P).T)      # [128, wpc]
        nrow = np.ascontiguousarray(nv.reshape(1, wpc * P))    # [1, wpc*128]

        in_maps.append({
            "hb": hb,
            "hwin_t": hwin_t,
            "nwin": nwin,
            "nrow": nrow,
            "cm_t0": cm_t0,
            "cm_t1": cm_t1,
            "idxw": np.ascontiguousarray(idxw),
            "meta2": np.ascontiguousarray(
                np.stack([slots_dstl, slots_nsrc], axis=1)
                .reshape(2 * T, P).T.astype(np.float32)),
            "r": np.asarray(r, np.float32),
            "Wm": np.asarray(W_msg, np.float32),
            "Wo": np.asarray(W, np.float32),
            "bvec": np.asarray(b, np.float32).reshape(1, D),
        })
    return struct, in_maps


def _unshard(outs, st):
    """[128, wpc*128] bf16 per core -> [N, 128] f32."""
    rows = []
    wpc, D = st["wpc"], st["D"]
    for o in outs:
        of = np.asarray(o).astype(np.float32)
        rows.append(of.reshape(P, wpc, D).transpose(1, 0, 2).reshape(wpc * P, D))
    return np.concatenate(rows, axis=0)[:st["N"]]


# ---------------------------------------------------------------------------
# Device program
# ---------------------------------------------------------------------------

def _build(st):
    NP_, D, R, wpc, T = st["NP"], st["D"], st["R"], st["wpc"], st["T"]
    lo_split = st["lo_split"]
    gm, gchunk = st["gm"], st["gchunk"]
    R1 = R - P                  # rows in the second relation chunk (72)

    nc = bacc.Bacc("TRN2", target_bir_lowering=False, debug=False,
                   dynamic_dma_scratch_size=16 * gchunk * P)

    hb = nc.declare_dram_parameter("hb", [NP_, D], BF16, isOutput=False)
    hwin_t = nc.declare_dram_parameter("hwin_t", [P, wpc * D], FP8,
                                       isOutput=False)
    nwin = nc.declare_dram_parameter("nwin", [P, wpc], FP32, isOutput=False)
    nrow = nc.declare_dram_parameter("nrow", [1, wpc * P], FP32,
                                     isOutput=False)
    cm_t0 = nc.declare_dram_parameter("cm_t0", [P, wpc * P], FP8,
                                      isOutput=False)
    cm_t1 = nc.declare_dram_parameter("cm_t1", [R1, wpc * P], FP8,
                                      isOutput=False)
    idxw = nc.declare_dram_parameter("idxw", [P, 8 * T], I16, isOutput=False)
    meta2 = nc.declare_dram_parameter("meta2", [P, 2 * T], FP32,
                                      isOutput=False)
    r_in = nc.declare_dram_parameter("r", [R, D], FP32, isOutput=False)
    Wm_in = nc.declare_dram_parameter("Wm", [D, D], FP32, isOutput=False)
    Wo_in = nc.declare_dram_parameter("Wo", [D, D], FP32, isOutput=False)
    b_in = nc.declare_dram_parameter("bvec", [1, D], FP32, isOutput=False)
    out = nc.declare_dram_parameter("out", [P, wpc * D], BF16, isOutput=True)

    with tile.TileContext(nc) as tc:
        with (
            tc.tile_pool(name="const", bufs=1) as cst,
            tc.tile_pool(name="meta", bufs=1) as meta,
            tc.tile_pool(name="xg", bufs=2) as xgp,
            tc.tile_pool(name="sm", bufs=6) as smp,
            tc.tile_pool(name="wn", bufs=3) as wnp,
            tc.tile_pool(name="pw", bufs=6, space="PSUM") as pwp,
            tc.tile_pool(name="po", bufs=2, space="PSUM") as pop,
        ):
            # ---- metadata heads first: unblock the first gather chunks ----
            t_head = min(T, 48)
            idx_s = meta.tile([P, 8 * T], I16)
            nc.sync.dma_start(idx_s[:, 0:8 * t_head], idxw[:, 0:8 * t_head])
            m2_s = meta.tile([P, 2 * T], FP32)
            nc.sync.dma_start(m2_s[:, 0:2 * t_head], meta2[:, 0:2 * t_head])

            # ---- persistent constants / metadata in SBUF ----
            iota_f = cst.tile([P, D], FP32)
            nc.gpsimd.iota(iota_f[:], pattern=[[1, D]], base=0,
                           channel_multiplier=0,
                           allow_small_or_imprecise_dtypes=True)
            iota_b = cst.tile([P, D], BF16)
            nc.vector.tensor_copy(iota_b[:], iota_f[:])

            Wm_b = cst.tile([P, D], BF16)
            Wo_b = cst.tile([P, D], FP8)
            b_b = cst.tile([1, D], BF16)

            wtmp = cst.tile([P, D], FP32, tag="wtmp")
            nc.scalar.dma_start(wtmp[:], Wm_in[:])
            nc.vector.tensor_copy(Wm_b[:], wtmp[:])
            wtmp2 = cst.tile([P, D], FP32, tag="wtmp2")
            nc.scalar.dma_start(wtmp2[:], Wo_in[:])
            nc.vector.tensor_copy(Wo_b[:], wtmp2[:])
            btmp = cst.tile([1, D], FP32, tag="btmp")
            nc.scalar.dma_start(btmp[:], b_in[:])
            nc.vector.tensor_copy(b_b[:], btmp[:])

            # negated relation table chunks (lhsT for the C correction)
            rtmp = cst.tile([P, D], FP32, tag="rtmp")
            nc.scalar.dma_start(rtmp[:], r_in[0:P, :])
            nr0 = cst.tile([P, D], BF16)
            nc.scalar.activation(nr0[:], rtmp[:],
                                 mybir.ActivationFunctionType.Copy,
                                 scale=-1.0)
            rtmp2 = cst.tile([R1, D], FP32, tag="rtmp2")
            nc.scalar.dma_start(rtmp2[:], r_in[P:R, :])
            nr1 = cst.tile([R1, D], BF16)
            nc.scalar.activation(nr1[:], rtmp2[:],
                                 mybir.ActivationFunctionType.Copy,
                                 scale=-1.0)

            # norm column tiles + 1/norm in row layout (for the bias matmul)
            nwin_s = cst.tile([P, wpc], FP32, tag="nwin_s")
            nc.scalar.dma_start(nwin_s[:], nwin[:])
            nrow_s = cst.tile([1, wpc * P], FP32, tag="nrow_s")
            nc.scalar.dma_start(nrow_s[:], nrow[:])
            nc.vector.tensor_scalar_max(nrow_s[:], nrow_s[:], 1e-6)
            nc.vector.reciprocal(nrow_s[:], nrow_s[:])
            invn_row = cst.tile([1, wpc * P], BF16, tag="invn_row")
            nc.vector.tensor_copy(invn_row[:], nrow_s[:])

            # metadata tails after the bulk prologue streams
            if t_head < T:
                nc.sync.dma_start(idx_s[:, 8 * t_head:], idxw[:, 8 * t_head:])
                nc.scalar.dma_start(m2_s[:, 2 * t_head:],
                                    meta2[:, 2 * t_head:])
            cm0_8 = meta.tile([P, wpc * P], FP8, tag="cm0_8")
            nc.scalar.dma_start(cm0_8[:], cm_t0[:])
            cm0_s = meta.tile([P, wpc * P], BF16)
            nc.scalar.activation(cm0_s[:], cm0_8[:],
                                 mybir.ActivationFunctionType.Copy)
            cm1_8 = meta.tile([R1, wpc * P], FP8, tag="cm1_8")
            nc.scalar.dma_start(cm1_8[:], cm_t1[:])
            cm1_s = meta.tile([R1, wpc * P], BF16)
            nc.scalar.activation(cm1_s[:], cm1_8[:],
                                 mybir.ActivationFunctionType.Copy)
            hw_all = meta.tile([P, wpc * D], FP8, tag="hw_all")
            nc.scalar.dma_start(hw_all[:], hwin_t[:])
            out_all = meta.tile([P, wpc * D], BF16)

            hb_lo = hb[0:lo_split, :]
            hb_hi = hb[lo_split:NP_, :]

            lo_t, hi_t = st["lo_tiles"], st["hi_tiles"]

            def window_epilogue(w, pw):
                """C correction + output matmuls + fused norm/relu."""
                nc.tensor.matmul(pw[:], lhsT=nr0[:],
                                 rhs=cm0_s[:, w * P:(w + 1) * P],
                                 start=False, stop=False,
                                 skip_group_check=True)
                nc.tensor.matmul(pw[:], lhsT=nr1[:],
                                 rhs=cm1_s[:, w * P:(w + 1) * P],
                                 start=False, stop=True,
                                 skip_group_check=True)
                segT_b = wnp.tile([P, D], BF16, tag="segT")
                nc.scalar.activation(segT_b[:], pw[:],
                                     mybir.ActivationFunctionType.Copy)
                op_ = pop.tile([P, D], FP32, tag="op")
                nc.tensor.matmul(op_[:], lhsT=segT_b[:], rhs=Wm_b[:],
                                 start=True, stop=False)
                nc.tensor.matmul(op_[:], lhsT=hw_all[:, w * D:(w + 1) * D],
                                 rhs=Wo_b[:], start=False, stop=False)
                nc.tensor.matmul(op_[:],
                                 lhsT=invn_row[0:1, w * P:(w + 1) * P],
                                 rhs=b_b[0:1, :], start=False, stop=True)
                nc.scalar.activation(out_all[:, w * D:(w + 1) * D], op_[:],
                                     mybir.ActivationFunctionType.Relu,
                                     scale=nwin_s[:, w:w + 1])

            # ---- main loop over groups ----
            for gi, ws in enumerate(st["groups"]):
                t0, ntt, chunks = st["group_meta"][gi]
                xg = xgp.tile([P, gm * D], BF16, tag="xg")
                xg3 = xg[:].rearrange("p (c e) -> p c e", e=D)
                for (rel_t0, n, lo_flag) in chunks:
                    tbl = hb_lo if lo_flag else hb_hi
                    nc.gpsimd.dma_gather(
                        out_ap=xg3[:, rel_t0:rel_t0 + n, :], in_ap=tbl,
                        idxs_ap=idx_s[:, 8 * (t0 + rel_t0):
                                      8 * (t0 + rel_t0 + n)],
                        num_idxs=n * P, num_idxs_reg=n * P,
                        elem_size=D)

                pw_of = {}
                remaining = {}
                for w in ws:
                    pw_of[w] = pwp.tile([P, D], FP32, tag="pw",
                                        name=f"pw_g{gi}_w{w}")
                    remaining[w] = lo_t[w] + hi_t[w]
                started = set()
                for tt in range(ntt):
                    ti = t0 + tt
                    w = st["tile_order"][ti][0]
                    # scaled one-hot: S[e, j] = (dstl_e == j) * nsrc_e
                    s_t = smp.tile([P, P], BF16, tag="s")
                    nc.vector.tensor_scalar(
                        out=s_t[:], in0=iota_b[:],
                        scalar1=m2_s[:, 2 * ti:2 * ti + 1],
                        scalar2=m2_s[:, 2 * ti + 1:2 * ti + 2],
                        op0=mybir.AluOpType.is_equal,
                        op1=mybir.AluOpType.mult)
                    first = w not in started
                    started.add(w)
                    remaining[w] -= 1
                    # psum_w[f, p] += X_t[slot, f]^T. S_t[slot, p]
                    nc.tensor.matmul(pw_of[w][:], lhsT=xg3[:, tt, :],
                                     rhs=s_t[:],
                                     start=first, stop=False,
                                     skip_group_check=True)
                    if remaining[w] == 0:
                        window_epilogue(w, pw_of[w])



            bounds = [0, 14, 28, 38, 44, 47, 48, wpc]
            for si, (o0, o1) in enumerate(zip(bounds[:-1], bounds[1:])):
                eng = nc.sync if si % 2 == 0 else nc.scalar
                eng.dma_start(out[:, o0 * D:o1 * D],
                              out_all[:, o0 * D:o1 * D])

    nc.compile()
    return nc


# ---------------------------------------------------------------------------
# Public entry
# ---------------------------------------------------------------------------

def _run(inputs, trace=False):
    st, in_maps = _prep(**inputs)
    nc = _build(st)
    res = run_bass_kernel_spmd(nc, in_maps, list(range(N_CORES)), trace=trace)
    full = _unshard([res.results[i]["out"] for i in range(N_CORES)], st)
    return np.ascontiguousarray(full, dtype=np.float32), res


def kernel(**inputs):
    out, _ = _run(inputs, trace=False)
    return out


def kernel_traced(**inputs):
    return _run(inputs, trace=True)


# revision 20
# speedup vs baseline: 1.0027x; 1.0027x over previous
"""CompGCN layer on 8 Trainium2 NeuronCores.

Reference computation:
    hn  = h * norm
    msg = (hn[src] - r[rel]) @ W_msg
    agg = segment_sum(msg, dst, N) * norm
    out = relu(hn @ W + agg + b)

Algebraic rewrite (matmul distributes over segment_sum):
    seg  = segment_sum(h[src]*norm[src], dst) - C @ r
    out  = relu(diag(norm) @ (h @ W + seg @ W_msg) + b)
where C[n,k] = #edges(dst=n, rel=k) is an integer histogram of the edge
structure (host-side index preprocessing, like the edge sort itself).
This turns the E x D x D per-edge matmul into an N x D x D one and the
scatter into one-hot matmuls accumulated in PSUM.

Sharding: edges are partitioned by 128-node destination windows; core i owns
49 consecutive windows and produces those output rows (no collectives).
h is replicated (as a bf16 table) so each core can gather arbitrary rows.

Device pipeline per 128-edge tile (edges pre-grouped by dst window on host):
    X  = dma_gather(hb, src)             # [128e, 128f] bf16 rows from HBM
    S  = (iota==dstl) * nsrc             # ONE fused DVE op -> scaled one-hot
    psum_w[f, p] += X.T @ S              # accumulates seg^T directly
Per-window epilogue (transpose-free: cmat/hwin shipped pre-transposed):
    psum_w  += (-r).T @ C^T chunks       # relation correction
    segT     = bf16(psum_w)              # one ACT copy
    op       = segT.T@Wm + hwinT.T@Wo + invnorm (x) b   # three matmuls
    out_w    = relu(norm * op)           # ACT with per-partition scale
(relu(norm*M + b) == relu(norm*(M + b/norm)) since norm > 0; the bias
enters as a rank-1 invnorm (x) b matmul, numerically safe as the 1/norm
rounding cancels when rescaled by norm.)
"""

import math
import numpy as np

from concourse import bass, bacc, mybir
from concourse import tile
from concourse.bass_utils import run_bass_kernel_spmd

FP32 = mybir.dt.float32
BF16 = mybir.dt.bfloat16
I16 = mybir.dt.int16

BF16_NP = np.dtype(mybir.dt.np(BF16))
FP8 = mybir.dt.float8e4
FP8_NP = np.dtype(mybir.dt.np(FP8))

P = 128          # partitions / window size / feature dim
N_CORES = 8


# ---------------------------------------------------------------------------
# Host-side preprocessing: index/layout work (sort, pad, wrap, integer
# histograms, dtype/layout prep of inputs). Per-edge scalar metadata
# (norm[src]) is gathered host-side; all math happens on device.
# ---------------------------------------------------------------------------

def _wrap16(idx_flat):
    """dma_gather index layout: i -> [partition i%16, col i//16], replicated
    to 128 partitions (8 Q7 cores each read one 16-row stripe)."""
    n = idx_flat.shape[0]
    assert n % 16 == 0
    w = idx_flat.reshape(n // 16, 16).T          # [16, n/16]
    return np.tile(w, (8, 1)).astype(np.int16)   # [128, n/16]


def _prep(h, r, norm, src, dst, rel, W_msg, W, b,
          n_cores=N_CORES, lo_split=32768, group_w=6, gchunk=8):
    N, D = h.shape
    R = r.shape[0]
    assert D == P
    RC = math.ceil(R / P)
    assert RC == 2

    NP_ = ((N + P - 1) // P) * P                 # padded node count
    n_win = NP_ // P                             # total windows
    wpc = (n_win + n_cores - 1) // n_cores       # windows per core (uniform)

    norm1 = np.asarray(norm).reshape(-1).astype(np.float32)
    src = np.asarray(src).astype(np.int64)
    dst = np.asarray(dst).astype(np.int64)
    rel = np.asarray(rel).astype(np.int64)

    win = dst // P                               # global window of each edge
    core = np.minimum(win // wpc, n_cores - 1)
    is_lo = src < lo_split

    # per-core per-window edge counts -> shared tile counts (max over cores)
    lo_cnt = np.zeros((n_cores, wpc), np.int64)
    hi_cnt = np.zeros((n_cores, wpc), np.int64)
    for c in range(n_cores):
        m = core == c
        wl = win[m] - c * wpc
        l = is_lo[m]
        np.add.at(lo_cnt[c], wl[l], 1)
        np.add.at(hi_cnt[c], wl[~l], 1)

    lo_tiles = np.maximum(1, np.ceil(lo_cnt.max(0) / P).astype(np.int64))
    hi_tiles = np.maximum(1, np.ceil(hi_cnt.max(0) / P).astype(np.int64))

    groups = [list(range(g, min(g + group_w, wpc)))
              for g in range(0, wpc, group_w)]

    # tile order (same for every core): per group, lo tiles then hi tiles.
    # gather chunks of <= gchunk tiles; a chunk never crosses the lo/hi
    # boundary (different tables).
    tile_order = []          # list of (window, is_lo)
    group_meta = []          # per group: (t0, ntiles, [(rel_t0, n, is_lo)])
    t = 0
    for ws in groups:
        t0 = t
        chunks = []
        for lo_flag, tcounts in ((True, lo_tiles), (False, hi_tiles)):
            seg0 = t
            for w in ws:
                for _ in range(int(tcounts[w])):
                    tile_order.append((w, lo_flag))
                    t += 1
            n_seg = t - seg0
            c = 0
            while c < n_seg:
                ce = min(c + gchunk, n_seg)
                chunks.append((seg0 - t0 + c, ce - c, lo_flag))
                c = ce
        group_meta.append((t0, t - t0, chunks))
    T = t                                       # total tiles per core
    gm = max(n for (_, n, _) in group_meta)     # max tiles per group

    struct = dict(N=N, NP=NP_, D=D, R=R, n_win=n_win, wpc=wpc,
                  lo_split=lo_split, groups=groups,
                  lo_tiles=[int(x) for x in lo_tiles],
                  hi_tiles=[int(x) for x in hi_tiles],
                  tile_order=tile_order, group_meta=group_meta,
                  T=T, gm=gm, gchunk=gchunk)

    h_np = np.asarray(h, np.float32)
    hb = np.zeros((NP_, D), BF16_NP)
    hb[:N] = h_np.astype(BF16_NP)                # bf16 gather table

    tile_ids = {}
    for ti, (w, lo) in enumerate(tile_order):
        tile_ids.setdefault((w, lo), []).append(ti)

    in_maps = []
    for c in range(n_cores):
        m = np.nonzero(core == c)[0]
        wl = win[m] - c * wpc
        slots_idx = np.zeros((T, P), np.int32)       # gather row index
        slots_dstl = np.full((T, P), P, np.float32)  # 128 sentinel -> S col off
        slots_nsrc = np.zeros((T, P), np.float32)

        fill = dict.fromkeys(tile_ids, 0)
        e_lo = is_lo[m]
        e_src = src[m]
        e_dstl = (dst[m] % P).astype(np.float32)
        e_nsrc = norm1[src[m]]
        for j in range(m.shape[0]):
            k = (int(wl[j]), bool(e_lo[j]))
            f = fill[k]
            ti = tile_ids[k][f // P]
            pos = f % P
            fill[k] = f + 1
            s = int(e_src[j])
            slots_idx[ti, pos] = s if e_lo[j] else s - lo_split
            slots_dstl[ti, pos] = e_dstl[j]
            slots_nsrc[ti, pos] = e_nsrc[j]

        idx_cols = []
        for (t0, ntt, chunks) in group_meta:
            for (rel_t0, n, _) in chunks:
                a = t0 + rel_t0
                idx_cols.append(_wrap16(slots_idx[a:a + n].reshape(-1)))
        idxw = np.concatenate(idx_cols, axis=1)      # [128, 8T]

        # integer (dst, rel) histogram for this core's windows, bf16-exact,
        # shipped pre-transposed: cmat_tk[k, w*128 + p] = C[w*128 + p, k']
        base = c * wpc * P
        cmat = np.zeros(wpc * P * R, np.int64)
        np.add.at(cmat, (dst[m] - base) * R + rel[m], 1)
        assert cmat.max() <= 16, "C counts exceed fp8-exact range"
        cmat = cmat.reshape(wpc * P, R)              # [wpc*128, R]
        cm_t0 = np.ascontiguousarray(cmat[:, :P].T.astype(FP8_NP))
        cm_t1 = np.ascontiguousarray(cmat[:, P:R].T.astype(FP8_NP))

        # own node rows shipped transposed: hwin_t[f, w*128+p] = h[base+...]
        own_n = min(max(N - base, 0), wpc * P)
        hwin = np.zeros((wpc * P, D), np.float32)
        nv = np.ones(wpc * P, np.float32)            # pad norm = 1 (finite inv)
        if own_n > 0:
            hwin[:own_n] = h_np[base:base + own_n]
            nv[:own_n] = norm1[base:base + own_n]
        hwin_t = np.ascontiguousarray(hwin.T.astype(FP8_NP))   # [128, wpc*128]
        nwin = np.ascontiguousarray(nv.reshape(wpc, P).T)      # [128, wpc]
        nrow = np.ascontiguousarray(nv.reshape(1, wpc * P))    # [1, wpc*128]

        in_maps.append({
            "hb": hb,
            "hwin_t": hwin_t,
            "nwin": nwin,
            "nrow": nrow,
            "cm_t0": cm_t0,
            "cm_t1": cm_t1,
            "idxw": np.ascontiguousarray(idxw),
            "meta2": np.ascontiguousarray(
                np.stack([slots_dstl, slots_nsrc], axis=1)
                .reshape(2 * T, P).T.astype(np.float32)),
            "r": np.asarray(r, np.float32),
            "Wm": np.asarray(W_msg, np.float32),
            "Wo": np.asarray(W, np.float32),
            "bvec": np.asarray(b, np.float32).reshape(1, D),
        })
    return struct, in_maps


def _unshard(outs, st):
    """[128, wpc*128] bf16 per core -> [N, 128] f32."""
    rows = []
    wpc, D = st["wpc"], st["D"]
    for o in outs:
        of = np.asarray(o).astype(np.float32)
        rows.append(of.reshape(P, wpc, D).transpose(1, 0, 2).reshape(wpc * P, D))
    return np.concatenate(rows, axis=0)[:st["N"]]


# ---------------------------------------------------------------------------
# Device program
# ---------------------------------------------------------------------------

def _build(st):
    NP_, D, R, wpc, T = st["NP"], st["D"], st["R"], st["wpc"], st["T"]
    lo_split = st["lo_split"]
    gm, gchunk = st["gm"], st["gchunk"]
    R1 = R - P                  # rows in the second relation chunk (72)

    nc = bacc.Bacc("TRN2", target_bir_lowering=False, debug=False,
                   dynamic_dma_scratch_size=16 * gchunk * P)

    hb = nc.declare_dram_parameter("hb", [NP_, D], BF16, isOutput=False)
    hwin_t = nc.declare_dram_parameter("hwin_t", [P, wpc * D], FP8,
                                       isOutput=False)
    nwin = nc.declare_dram_parameter("nwin", [P, wpc], FP32, isOutput=False)
    nrow = nc.declare_dram_parameter("nrow", [1, wpc * P], FP32,
                                     isOutput=False)
    cm_t0 = nc.declare_dram_parameter("cm_t0", [P, wpc * P], FP8,
                                      isOutput=False)
    cm_t1 = nc.declare_dram_parameter("cm_t1", [R1, wpc * P], FP8,
                                      isOutput=False)
    idxw = nc.declare_dram_parameter("idxw", [P, 8 * T], I16, isOutput=False)
    meta2 = nc.declare_dram_parameter("meta2", [P, 2 * T], FP32,
                                      isOutput=False)
    r_in = nc.declare_dram_parameter("r", [R, D], FP32, isOutput=False)
    Wm_in = nc.declare_dram_parameter("Wm", [D, D], FP32, isOutput=False)
    Wo_in = nc.declare_dram_parameter("Wo", [D, D], FP32, isOutput=False)
    b_in = nc.declare_dram_parameter("bvec", [1, D], FP32, isOutput=False)
    out = nc.declare_dram_parameter("out", [P, wpc * D], BF16, isOutput=True)

    with tile.TileContext(nc) as tc:
        with (
            tc.tile_pool(name="const", bufs=1) as cst,
            tc.tile_pool(name="meta", bufs=1) as meta,
            tc.tile_pool(name="xg", bufs=2) as xgp,
            tc.tile_pool(name="sm", bufs=6) as smp,
            tc.tile_pool(name="wn", bufs=3) as wnp,
            tc.tile_pool(name="pw", bufs=6, space="PSUM") as pwp,
            tc.tile_pool(name="po", bufs=2, space="PSUM") as pop,
        ):
            # ---- metadata heads first: unblock the first gather chunks ----
            t_head = min(T, max(32, T // 8))
            idx_s = meta.tile([P, 8 * T], I16)
            nc.sync.dma_start(idx_s[:, 0:8 * t_head], idxw[:, 0:8 * t_head])
            m2_s = meta.tile([P, 2 * T], FP32)
            nc.sync.dma_start(m2_s[:, 0:2 * t_head], meta2[:, 0:2 * t_head])

            # ---- persistent constants / metadata in SBUF ----
            iota_f = cst.tile([P, D], FP32)
            nc.gpsimd.iota(iota_f[:], pattern=[[1, D]], base=0,
                           channel_multiplier=0,
                           allow_small_or_imprecise_dtypes=True)
            iota_b = cst.tile([P, D], BF16)
            nc.vector.tensor_copy(iota_b[:], iota_f[:])

            Wm_b = cst.tile([P, D], BF16)
            Wo_b = cst.tile([P, D], FP8)
            b_b = cst.tile([1, D], BF16)

            wtmp = cst.tile([P, D], FP32, tag="wtmp")
            nc.scalar.dma_start(wtmp[:], Wm_in[:])
            nc.vector.tensor_copy(Wm_b[:], wtmp[:])
            wtmp2 = cst.tile([P, D], FP32, tag="wtmp2")
            nc.scalar.dma_start(wtmp2[:], Wo_in[:])
            nc.vector.tensor_copy(Wo_b[:], wtmp2[:])
            btmp = cst.tile([1, D], FP32, tag="btmp")
            nc.scalar.dma_start(btmp[:], b_in[:])
            nc.vector.tensor_copy(b_b[:], btmp[:])

            # negated relation table chunks (lhsT for the C correction)
            rtmp = cst.tile([P, D], FP32, tag="rtmp")
            nc.scalar.dma_start(rtmp[:], r_in[0:P, :])
            nr0 = cst.tile([P, D], BF16)
            nc.scalar.activation(nr0[:], rtmp[:],
                                 mybir.ActivationFunctionType.Copy,
                                 scale=-1.0)
            rtmp2 = cst.tile([R1, D], FP32, tag="rtmp2")
            nc.scalar.dma_start(rtmp2[:], r_in[P:R, :])
            nr1 = cst.tile([R1, D], BF16)
            nc.scalar.activation(nr1[:], rtmp2[:],
                                 mybir.ActivationFunctionType.Copy,
                                 scale=-1.0)

            # norm column tiles + 1/norm in row layout (for the bias matmul)
            nwin_s = cst.tile([P, wpc], FP32, tag="nwin_s")
            nc.scalar.dma_start(nwin_s[:], nwin[:])
            nrow_s = cst.tile([1, wpc * P], FP32, tag="nrow_s")
            nc.scalar.dma_start(nrow_s[:], nrow[:])
            nc.vector.tensor_scalar_max(nrow_s[:], nrow_s[:], 1e-6)
            nc.vector.reciprocal(nrow_s[:], nrow_s[:])
            invn_row = cst.tile([1, wpc * P], BF16, tag="invn_row")
            nc.vector.tensor_copy(invn_row[:], nrow_s[:])

            # metadata tails after the bulk prologue streams
            if t_head < T:
                nc.sync.dma_start(idx_s[:, 8 * t_head:], idxw[:, 8 * t_head:])
                nc.sync.dma_start(m2_s[:, 2 * t_head:], meta2[:, 2 * t_head:])
            cm0_8 = meta.tile([P, wpc * P], FP8, tag="cm0_8")
            nc.scalar.dma_start(cm0_8[:], cm_t0[:])
            cm0_s = meta.tile([P, wpc * P], BF16)
            nc.scalar.activation(cm0_s[:], cm0_8[:],
                                 mybir.ActivationFunctionType.Copy)
            cm1_8 = meta.tile([R1, wpc * P], FP8, tag="cm1_8")
            nc.scalar.dma_start(cm1_8[:], cm_t1[:])
            cm1_s = meta.tile([R1, wpc * P], BF16)
            nc.scalar.activation(cm1_s[:], cm1_8[:],
                                 mybir.ActivationFunctionType.Copy)
            hw_all = meta.tile([P, wpc * D], FP8, tag="hw_all")
            nc.scalar.dma_start(hw_all[:], hwin_t[:])
            out_all = meta.tile([P, wpc * D], BF16)

            hb_lo = hb[0:lo_split, :]
            hb_hi = hb[lo_split:NP_, :]

            lo_t, hi_t = st["lo_tiles"], st["hi_tiles"]

            def window_epilogue(w, pw):
                """C correction + output matmuls + fused norm/relu."""
                nc.tensor.matmul(pw[:], lhsT=nr0[:],
                                 rhs=cm0_s[:, w * P:(w + 1) * P],
                                 start=False, stop=False,
                                 skip_group_check=True)
                nc.tensor.matmul(pw[:], lhsT=nr1[:],
                                 rhs=cm1_s[:, w * P:(w + 1) * P],
                                 start=False, stop=True,
                                 skip_group_check=True)
                segT_b = wnp.tile([P, D], BF16, tag="segT")
                nc.scalar.activation(segT_b[:], pw[:],
                                     mybir.ActivationFunctionType.Copy)
                op_ = pop.tile([P, D], FP32, tag="op")
                nc.tensor.matmul(op_[:], lhsT=segT_b[:], rhs=Wm_b[:],
                                 start=True, stop=False)
                nc.tensor.matmul(op_[:], lhsT=hw_all[:, w * D:(w + 1) * D],
                                 rhs=Wo_b[:], start=False, stop=False)
                nc.tensor.matmul(op_[:],
                                 lhsT=invn_row[0:1, w * P:(w + 1) * P],
                                 rhs=b_b[0:1, :], start=False, stop=True)
                nc.scalar.activation(out_all[:, w * D:(w + 1) * D], op_[:],
                                     mybir.ActivationFunctionType.Relu,
                                     scale=nwin_s[:, w:w + 1])

            # ---- main loop over groups ----
            for gi, ws in enumerate(st["groups"]):
                t0, ntt, chunks = st["group_meta"][gi]
                xg = xgp.tile([P, gm * D], BF16, tag="xg")
                xg3 = xg[:].rearrange("p (c e) -> p c e", e=D)
                for (rel_t0, n, lo_flag) in chunks:
                    tbl = hb_lo if lo_flag else hb_hi
                    nc.gpsimd.dma_gather(
                        out_ap=xg3[:, rel_t0:rel_t0 + n, :], in_ap=tbl,
                        idxs_ap=idx_s[:, 8 * (t0 + rel_t0):
                                      8 * (t0 + rel_t0 + n)],
                        num_idxs=n * P, num_idxs_reg=n * P,
                        elem_size=D)

                pw_of = {}
                remaining = {}
                for w in ws:
                    pw_of[w] = pwp.tile([P, D], FP32, tag="pw",
                                        name=f"pw_g{gi}_w{w}")
                    remaining[w] = lo_t[w] + hi_t[w]
                started = set()
                for tt in range(ntt):
                    ti = t0 + tt
                    w = st["tile_order"][ti][0]
                    # scaled one-hot: S[e, j] = (dstl_e == j) * nsrc_e
                    s_t = smp.tile([P, P], BF16, tag="s")
                    nc.vector.tensor_scalar(
                        out=s_t[:], in0=iota_b[:],
                        scalar1=m2_s[:, 2 * ti:2 * ti + 1],
                        scalar2=m2_s[:, 2 * ti + 1:2 * ti + 2],
                        op0=mybir.AluOpType.is_equal,
                        op1=mybir.AluOpType.mult)
                    first = w not in started
                    started.add(w)
                    remaining[w] -= 1
                    # psum_w[f, p] += X_t[slot, f]^T. S_t[slot, p]
                    nc.tensor.matmul(pw_of[w][:], lhsT=xg3[:, tt, :],
                                     rhs=s_t[:],
                                     start=first, stop=False,
                                     skip_group_check=True)
                    if remaining[w] == 0:
                        window_epilogue(w, pw_of[w])



            bounds = [0, 14, 28, 38, 44, 47, 48, wpc]
            for si, (o0, o1) in enumerate(zip(bounds[:-1], bounds[1:])):
                eng = nc.sync if si % 2 == 0 else nc.scalar
                eng.dma_start(out[:, o0 * D:o1 * D],
                              out_all[:, o0 * D:o1 * D])

    nc.compile()
    return nc


# ---------------------------------------------------------------------------
# Public entry
# ---------------------------------------------------------------------------

def _run(inputs, trace=False):
    st, in_maps = _prep(**inputs)
    nc = _build(st)
    res = run_bass_kernel_spmd(nc, in_maps, list(range(N_CORES)), trace=trace)
    full = _unshard([res.results[i]["out"] for i in range(N_CORES)], st)
    return np.ascontiguousarray(full, dtype=np.float32), res


def kernel(**inputs):
    out, _ = _run(inputs, trace=False)
    return out


def kernel_traced(**inputs):
    return _run(inputs, trace=True)


# revision 21
# speedup vs baseline: 1.0036x; 1.0009x over previous
"""CompGCN layer on 8 Trainium2 NeuronCores.

Reference computation:
    hn  = h * norm
    msg = (hn[src] - r[rel]) @ W_msg
    agg = segment_sum(msg, dst, N) * norm
    out = relu(hn @ W + agg + b)

Algebraic rewrite (matmul distributes over segment_sum):
    seg  = segment_sum(h[src]*norm[src], dst) - C @ r
    out  = relu(diag(norm) @ (h @ W + seg @ W_msg) + b)
where C[n,k] = #edges(dst=n, rel=k) is an integer histogram of the edge
structure (host-side index preprocessing, like the edge sort itself).
This turns the E x D x D per-edge matmul into an N x D x D one and the
scatter into one-hot matmuls accumulated in PSUM.

Sharding: edges are partitioned by 128-node destination windows; core i owns
49 consecutive windows and produces those output rows (no collectives).
h is replicated (as a bf16 table) so each core can gather arbitrary rows.

Device pipeline per 128-edge tile (edges pre-grouped by dst window on host):
    X  = dma_gather(hb, src)             # [128e, 128f] bf16 rows from HBM
    S  = (iota==dstl) * nsrc             # ONE fused DVE op -> scaled one-hot
    psum_w[f, p] += X.T @ S              # accumulates seg^T directly
Per-window epilogue (transpose-free: cmat/hwin shipped pre-transposed):
    psum_w  += (-r).T @ C^T chunks       # relation correction
    segT     = bf16(psum_w)              # one ACT copy
    op       = segT.T@Wm + hwinT.T@Wo + invnorm (x) b   # three matmuls
    out_w    = relu(norm * op)           # ACT with per-partition scale
(relu(norm*M + b) == relu(norm*(M + b/norm)) since norm > 0; the bias
enters as a rank-1 invnorm (x) b matmul, numerically safe as the 1/norm
rounding cancels when rescaled by norm.)
"""

import math
import numpy as np

from concourse import bass, bacc, mybir
from concourse import tile
from concourse.bass_utils import run_bass_kernel_spmd

FP32 = mybir.dt.float32
BF16 = mybir.dt.bfloat16
I16 = mybir.dt.int16

BF16_NP = np.dtype(mybir.dt.np(BF16))
FP8 = mybir.dt.float8e4
FP8_NP = np.dtype(mybir.dt.np(FP8))

P = 128          # partitions / window size / feature dim
N_CORES = 8


# ---------------------------------------------------------------------------
# Host-side preprocessing: index/layout work (sort, pad, wrap, integer
# histograms, dtype/layout prep of inputs). Per-edge scalar metadata
# (norm[src]) is gathered host-side; all math happens on device.
# ---------------------------------------------------------------------------

def _wrap16(idx_flat):
    """dma_gather index layout: i -> [partition i%16, col i//16], replicated
    to 128 partitions (8 Q7 cores each read one 16-row stripe)."""
    n = idx_flat.shape[0]
    assert n % 16 == 0
    w = idx_flat.reshape(n // 16, 16).T          # [16, n/16]
    return np.tile(w, (8, 1)).astype(np.int16)   # [128, n/16]


def _prep(h, r, norm, src, dst, rel, W_msg, W, b,
          n_cores=N_CORES, lo_split=32768, group_w=6, gchunk=8):
    N, D = h.shape
    R = r.shape[0]
    assert D == P
    RC = math.ceil(R / P)
    assert RC == 2

    NP_ = ((N + P - 1) // P) * P                 # padded node count
    n_win = NP_ // P                             # total windows
    wpc = (n_win + n_cores - 1) // n_cores       # windows per core (uniform)

    norm1 = np.asarray(norm).reshape(-1).astype(np.float32)
    src = np.asarray(src).astype(np.int64)
    dst = np.asarray(dst).astype(np.int64)
    rel = np.asarray(rel).astype(np.int64)

    win = dst // P                               # global window of each edge
    core = np.minimum(win // wpc, n_cores - 1)
    is_lo = src < lo_split

    # per-core per-window edge counts -> shared tile counts (max over cores)
    lo_cnt = np.zeros((n_cores, wpc), np.int64)
    hi_cnt = np.zeros((n_cores, wpc), np.int64)
    for c in range(n_cores):
        m = core == c
        wl = win[m] - c * wpc
        l = is_lo[m]
        np.add.at(lo_cnt[c], wl[l], 1)
        np.add.at(hi_cnt[c], wl[~l], 1)

    lo_tiles = np.maximum(1, np.ceil(lo_cnt.max(0) / P).astype(np.int64))
    hi_tiles = np.maximum(1, np.ceil(hi_cnt.max(0) / P).astype(np.int64))

    groups = [list(range(g, min(g + group_w, wpc)))
              for g in range(0, wpc, group_w)]

    # tile order (same for every core): per group, lo tiles then hi tiles.
    # gather chunks of <= gchunk tiles; a chunk never crosses the lo/hi
    # boundary (different tables).
    tile_order = []          # list of (window, is_lo)
    group_meta = []          # per group: (t0, ntiles, [(rel_t0, n, is_lo)])
    t = 0
    for gi_, ws in enumerate(groups):
        t0 = t
        chunks = []
        # micro-chunk the final group so its matmuls overlap its gathers
        # (its epilogue chain is the timeline tail)
        gch = gchunk if gi_ < len(groups) - 1 else 3
        for lo_flag, tcounts in ((True, lo_tiles), (False, hi_tiles)):
            seg0 = t
            for w in ws:
                for _ in range(int(tcounts[w])):
                    tile_order.append((w, lo_flag))
                    t += 1
            n_seg = t - seg0
            c = 0
            while c < n_seg:
                ce = min(c + gch, n_seg)
                chunks.append((seg0 - t0 + c, ce - c, lo_flag))
                c = ce
        group_meta.append((t0, t - t0, chunks))
    T = t                                       # total tiles per core
    gm = max(n for (_, n, _) in group_meta)     # max tiles per group

    struct = dict(N=N, NP=NP_, D=D, R=R, n_win=n_win, wpc=wpc,
                  lo_split=lo_split, groups=groups,
                  lo_tiles=[int(x) for x in lo_tiles],
                  hi_tiles=[int(x) for x in hi_tiles],
                  tile_order=tile_order, group_meta=group_meta,
                  T=T, gm=gm, gchunk=gchunk)

    h_np = np.asarray(h, np.float32)
    hb = np.zeros((NP_, D), BF16_NP)
    hb[:N] = h_np.astype(BF16_NP)                # bf16 gather table

    tile_ids = {}
    for ti, (w, lo) in enumerate(tile_order):
        tile_ids.setdefault((w, lo), []).append(ti)

    in_maps = []
    for c in range(n_cores):
        m = np.nonzero(core == c)[0]
        wl = win[m] - c * wpc
        slots_idx = np.zeros((T, P), np.int32)       # gather row index
        slots_dstl = np.full((T, P), P, np.float32)  # 128 sentinel -> S col off
        slots_nsrc = np.zeros((T, P), np.float32)

        fill = dict.fromkeys(tile_ids, 0)
        e_lo = is_lo[m]
        e_src = src[m]
        e_dstl = (dst[m] % P).astype(np.float32)
        e_nsrc = norm1[src[m]]
        for j in range(m.shape[0]):
            k = (int(wl[j]), bool(e_lo[j]))
            f = fill[k]
            ti = tile_ids[k][f // P]
            pos = f % P
            fill[k] = f + 1
            s = int(e_src[j])
            slots_idx[ti, pos] = s if e_lo[j] else s - lo_split
            slots_dstl[ti, pos] = e_dstl[j]
            slots_nsrc[ti, pos] = e_nsrc[j]

        idx_cols = []
        for (t0, ntt, chunks) in group_meta:
            for (rel_t0, n, _) in chunks:
                a = t0 + rel_t0
                idx_cols.append(_wrap16(slots_idx[a:a + n].reshape(-1)))
        idxw = np.concatenate(idx_cols, axis=1)      # [128, 8T]

        # integer (dst, rel) histogram for this core's windows, bf16-exact,
        # shipped pre-transposed: cmat_tk[k, w*128 + p] = C[w*128 + p, k']
        base = c * wpc * P
        cmat = np.zeros(wpc * P * R, np.int64)
        np.add.at(cmat, (dst[m] - base) * R + rel[m], 1)
        assert cmat.max() <= 16, "C counts exceed fp8-exact range"
        cmat = cmat.reshape(wpc * P, R)              # [wpc*128, R]
        cm_t0 = np.ascontiguousarray(cmat[:, :P].T.astype(FP8_NP))
        cm_t1 = np.ascontiguousarray(cmat[:, P:R].T.astype(FP8_NP))

        # own node rows shipped transposed: hwin_t[f, w*128+p] = h[base+...]
        own_n = min(max(N - base, 0), wpc * P)
        hwin = np.zeros((wpc * P, D), np.float32)
        nv = np.ones(wpc * P, np.float32)            # pad norm = 1 (finite inv)
        if own_n > 0:
            hwin[:own_n] = h_np[base:base + own_n]
            nv[:own_n] = norm1[base:base + own_n]
        hwin_t = np.ascontiguousarray(hwin.T.astype(FP8_NP))   # [128, wpc*128]
        nwin = np.ascontiguousarray(nv.reshape(wpc, P).T)      # [128, wpc]
        nrow = np.ascontiguousarray(nv.reshape(1, wpc * P))    # [1, wpc*128]

        in_maps.append({
            "hb": hb,
            "hwin_t": hwin_t,
            "nwin": nwin,
            "nrow": nrow,
            "cm_t0": cm_t0,
            "cm_t1": cm_t1,
            "idxw": np.ascontiguousarray(idxw),
            "meta2": np.ascontiguousarray(
                np.stack([slots_dstl, slots_nsrc], axis=1)
                .reshape(2 * T, P).T.astype(np.float32)),
            "r": np.asarray(r, np.float32),
            "Wm": np.asarray(W_msg, np.float32),
            "Wo": np.asarray(W, np.float32),
            "bvec": np.asarray(b, np.float32).reshape(1, D),
        })
    return struct, in_maps


def _unshard(outs, st):
    """[128, wpc*128] bf16 per core -> [N, 128] f32."""
    rows = []
    wpc, D = st["wpc"], st["D"]
    for o in outs:
        of = np.asarray(o).astype(np.float32)
        rows.append(of.reshape(P, wpc, D).transpose(1, 0, 2).reshape(wpc * P, D))
    return np.concatenate(rows, axis=0)[:st["N"]]


# ---------------------------------------------------------------------------
# Device program
# ---------------------------------------------------------------------------

def _build(st):
    NP_, D, R, wpc, T = st["NP"], st["D"], st["R"], st["wpc"], st["T"]
    lo_split = st["lo_split"]
    gm, gchunk = st["gm"], st["gchunk"]
    R1 = R - P                  # rows in the second relation chunk (72)

    nc = bacc.Bacc("TRN2", target_bir_lowering=False, debug=False,
                   dynamic_dma_scratch_size=16 * gchunk * P)

    hb = nc.declare_dram_parameter("hb", [NP_, D], BF16, isOutput=False)
    hwin_t = nc.declare_dram_parameter("hwin_t", [P, wpc * D], FP8,
                                       isOutput=False)
    nwin = nc.declare_dram_parameter("nwin", [P, wpc], FP32, isOutput=False)
    nrow = nc.declare_dram_parameter("nrow", [1, wpc * P], FP32,
                                     isOutput=False)
    cm_t0 = nc.declare_dram_parameter("cm_t0", [P, wpc * P], FP8,
                                      isOutput=False)
    cm_t1 = nc.declare_dram_parameter("cm_t1", [R1, wpc * P], FP8,
                                      isOutput=False)
    idxw = nc.declare_dram_parameter("idxw", [P, 8 * T], I16, isOutput=False)
    meta2 = nc.declare_dram_parameter("meta2", [P, 2 * T], FP32,
                                      isOutput=False)
    r_in = nc.declare_dram_parameter("r", [R, D], FP32, isOutput=False)
    Wm_in = nc.declare_dram_parameter("Wm", [D, D], FP32, isOutput=False)
    Wo_in = nc.declare_dram_parameter("Wo", [D, D], FP32, isOutput=False)
    b_in = nc.declare_dram_parameter("bvec", [1, D], FP32, isOutput=False)
    out = nc.declare_dram_parameter("out", [P, wpc * D], BF16, isOutput=True)

    with tile.TileContext(nc) as tc:
        with (
            tc.tile_pool(name="const", bufs=1) as cst,
            tc.tile_pool(name="meta", bufs=1) as meta,
            tc.tile_pool(name="xg", bufs=2) as xgp,
            tc.tile_pool(name="sm", bufs=6) as smp,
            tc.tile_pool(name="wn", bufs=3) as wnp,
            tc.tile_pool(name="pw", bufs=6, space="PSUM") as pwp,
            tc.tile_pool(name="po", bufs=2, space="PSUM") as pop,
        ):
            # ---- metadata heads first: unblock the first gather chunks ----
            t_head = min(T, max(32, T // 8))
            idx_s = meta.tile([P, 8 * T], I16)
            nc.sync.dma_start(idx_s[:, 0:8 * t_head], idxw[:, 0:8 * t_head])
            m2_s = meta.tile([P, 2 * T], FP32)
            nc.sync.dma_start(m2_s[:, 0:2 * t_head], meta2[:, 0:2 * t_head])

            # ---- persistent constants / metadata in SBUF ----
            iota_f = cst.tile([P, D], FP32)
            nc.gpsimd.iota(iota_f[:], pattern=[[1, D]], base=0,
                           channel_multiplier=0,
                           allow_small_or_imprecise_dtypes=True)
            iota_b = cst.tile([P, D], BF16)
            nc.vector.tensor_copy(iota_b[:], iota_f[:])

            Wm_b = cst.tile([P, D], BF16)
            Wo_b = cst.tile([P, D], FP8)
            b_b = cst.tile([1, D], BF16)

            wtmp = cst.tile([P, D], FP32, tag="wtmp")
            nc.scalar.dma_start(wtmp[:], Wm_in[:])
            nc.vector.tensor_copy(Wm_b[:], wtmp[:])
            wtmp2 = cst.tile([P, D], FP32, tag="wtmp2")
            nc.scalar.dma_start(wtmp2[:], Wo_in[:])
            nc.vector.tensor_copy(Wo_b[:], wtmp2[:])
            btmp = cst.tile([1, D], FP32, tag="btmp")
            nc.scalar.dma_start(btmp[:], b_in[:])
            nc.vector.tensor_copy(b_b[:], btmp[:])

            # negated relation table chunks (lhsT for the C correction)
            rtmp = cst.tile([P, D], FP32, tag="rtmp")
            nc.scalar.dma_start(rtmp[:], r_in[0:P, :])
            nr0 = cst.tile([P, D], BF16)
            nc.scalar.activation(nr0[:], rtmp[:],
                                 mybir.ActivationFunctionType.Copy,
                                 scale=-1.0)
            rtmp2 = cst.tile([R1, D], FP32, tag="rtmp2")
            nc.scalar.dma_start(rtmp2[:], r_in[P:R, :])
            nr1 = cst.tile([R1, D], BF16)
            nc.scalar.activation(nr1[:], rtmp2[:],
                                 mybir.ActivationFunctionType.Copy,
                                 scale=-1.0)

            # norm column tiles + 1/norm in row layout (for the bias matmul)
            nwin_s = cst.tile([P, wpc], FP32, tag="nwin_s")
            nc.scalar.dma_start(nwin_s[:], nwin[:])
            nrow_s = cst.tile([1, wpc * P], FP32, tag="nrow_s")
            nc.scalar.dma_start(nrow_s[:], nrow[:])
            nc.vector.tensor_scalar_max(nrow_s[:], nrow_s[:], 1e-6)
            nc.vector.reciprocal(nrow_s[:], nrow_s[:])
            invn_row = cst.tile([1, wpc * P], BF16, tag="invn_row")
            nc.vector.tensor_copy(invn_row[:], nrow_s[:])

            # metadata tails after the bulk prologue streams
            if t_head < T:
                nc.sync.dma_start(idx_s[:, 8 * t_head:], idxw[:, 8 * t_head:])
                nc.sync.dma_start(m2_s[:, 2 * t_head:], meta2[:, 2 * t_head:])
            cm0_8 = meta.tile([P, wpc * P], FP8, tag="cm0_8")
            nc.scalar.dma_start(cm0_8[:], cm_t0[:])
            cm0_s = meta.tile([P, wpc * P], BF16)
            nc.scalar.activation(cm0_s[:], cm0_8[:],
                                 mybir.ActivationFunctionType.Copy)
            cm1_8 = meta.tile([R1, wpc * P], FP8, tag="cm1_8")
            nc.scalar.dma_start(cm1_8[:], cm_t1[:])
            cm1_s = meta.tile([R1, wpc * P], BF16)
            nc.scalar.activation(cm1_s[:], cm1_8[:],
                                 mybir.ActivationFunctionType.Copy)
            hw_all = meta.tile([P, wpc * D], FP8, tag="hw_all")
            nc.scalar.dma_start(hw_all[:], hwin_t[:])
            out_all = meta.tile([P, wpc * D], BF16)

            hb_lo = hb[0:lo_split, :]
            hb_hi = hb[lo_split:NP_, :]

            lo_t, hi_t = st["lo_tiles"], st["hi_tiles"]

            def window_epilogue(w, pw):
                """C correction + output matmuls + fused norm/relu."""
                nc.tensor.matmul(pw[:], lhsT=nr0[:],
                                 rhs=cm0_s[:, w * P:(w + 1) * P],
                                 start=False, stop=False,
                                 skip_group_check=True)
                nc.tensor.matmul(pw[:], lhsT=nr1[:],
                                 rhs=cm1_s[:, w * P:(w + 1) * P],
                                 start=False, stop=True,
                                 skip_group_check=True)
                segT_b = wnp.tile([P, D], BF16, tag="segT")
                nc.scalar.activation(segT_b[:], pw[:],
                                     mybir.ActivationFunctionType.Copy)
                op_ = pop.tile([P, D], FP32, tag="op")
                nc.tensor.matmul(op_[:], lhsT=segT_b[:], rhs=Wm_b[:],
                                 start=True, stop=False)
                nc.tensor.matmul(op_[:], lhsT=hw_all[:, w * D:(w + 1) * D],
                                 rhs=Wo_b[:], start=False, stop=False)
                nc.tensor.matmul(op_[:],
                                 lhsT=invn_row[0:1, w * P:(w + 1) * P],
                                 rhs=b_b[0:1, :], start=False, stop=True)
                nc.scalar.activation(out_all[:, w * D:(w + 1) * D], op_[:],
                                     mybir.ActivationFunctionType.Relu,
                                     scale=nwin_s[:, w:w + 1])

            # ---- main loop over groups ----
            for gi, ws in enumerate(st["groups"]):
                t0, ntt, chunks = st["group_meta"][gi]
                xg = xgp.tile([P, gm * D], BF16, tag="xg")
                xg3 = xg[:].rearrange("p (c e) -> p c e", e=D)
                for (rel_t0, n, lo_flag) in chunks:
                    tbl = hb_lo if lo_flag else hb_hi
                    nc.gpsimd.dma_gather(
                        out_ap=xg3[:, rel_t0:rel_t0 + n, :], in_ap=tbl,
                        idxs_ap=idx_s[:, 8 * (t0 + rel_t0):
                                      8 * (t0 + rel_t0 + n)],
                        num_idxs=n * P, num_idxs_reg=n * P,
                        elem_size=D)

                pw_of = {}
                remaining = {}
                for w in ws:
                    pw_of[w] = pwp.tile([P, D], FP32, tag="pw",
                                        name=f"pw_g{gi}_w{w}")
                    remaining[w] = lo_t[w] + hi_t[w]
                started = set()
                for tt in range(ntt):
                    ti = t0 + tt
                    w = st["tile_order"][ti][0]
                    # scaled one-hot: S[e, j] = (dstl_e == j) * nsrc_e
                    s_t = smp.tile([P, P], BF16, tag="s")
                    nc.vector.tensor_scalar(
                        out=s_t[:], in0=iota_b[:],
                        scalar1=m2_s[:, 2 * ti:2 * ti + 1],
                        scalar2=m2_s[:, 2 * ti + 1:2 * ti + 2],
                        op0=mybir.AluOpType.is_equal,
                        op1=mybir.AluOpType.mult)
                    first = w not in started
                    started.add(w)
                    remaining[w] -= 1
                    # psum_w[f, p] += X_t[slot, f]^T. S_t[slot, p]
                    nc.tensor.matmul(pw_of[w][:], lhsT=xg3[:, tt, :],
                                     rhs=s_t[:],
                                     start=first, stop=False,
                                     skip_group_check=True)
                    if remaining[w] == 0:
                        window_epilogue(w, pw_of[w])



            bounds = [0, 14, 28, 38, 44, 47, 48, wpc]
            for si, (o0, o1) in enumerate(zip(bounds[:-1], bounds[1:])):
                eng = nc.sync if si % 2 == 0 else nc.scalar
                eng.dma_start(out[:, o0 * D:o1 * D],
                              out_all[:, o0 * D:o1 * D])

    nc.compile()
    return nc


# ---------------------------------------------------------------------------
# Public entry
# ---------------------------------------------------------------------------

def _run(inputs, trace=False):
    st, in_maps = _prep(**inputs)
    nc = _build(st)
    res = run_bass_kernel_spmd(nc, in_maps, list(range(N_CORES)), trace=trace)
    full = _unshard([res.results[i]["out"] for i in range(N_CORES)], st)
    return np.ascontiguousarray(full, dtype=np.float32), res


def kernel(**inputs):
    out, _ = _run(inputs, trace=False)
    return out


def kernel_traced(**inputs):
    return _run(inputs, trace=True)
